# revision 20
# baseline (speedup 1.0000x reference)
"""Cross-attention (S2Audio) Trainium2 Bass kernel.

Sharding: data-parallel over the clip batch B=8 -> one batch element per
NeuronCore.  Per core the kernel computes, for its batch element b:

  q = (audio_patch + pos_a) @ q_w.T + q_b          (1568, 768)
  k,v = (s_x_patch + pos_s) @ kv_w.T + kv_b        (1568, 768) each
  out = softmax(q k^T / sqrt(64)) v  per 12 heads  -> proj -> (1568, 768)

Host prep is layout/elementwise only: weight transposes, positional-embedding
combine + add (O(N*D)), bf16 casts, sharding slices.  All matmuls/softmax run
on device.

Performance-critical structure (v2):
  * The TRN2 PE clock-gates to 1.2 GHz (HAM K=4/8) whenever it idles; dense
    back-to-back matmul emission keeps it at 2.4 GHz.  All per-head serial
    work (softmax normalization) is OFF the PE queue: denominators come free
    from a ones-column in the PV matmul, reciprocals are batched per block on
    DVE ([12, nq] in one instruction), the partition-broadcast runs on the
    otherwise-idle GpSimd engine, and the final scale is an in-place DVE mul.
    The whole normalize + O-projection of block b-1 is software-pipelined
    into block b's head loop.
  * Scores matmuls have K=64 (head dim) -> 64x128 PE row tiling: the two
    heads of a pair live on SBUF partitions 0-63 / 64-127, their score
    matmuls are emitted interleaved (tile_position (0,0)/(64,0)) so they
    stream CONCURRENTLY through the two 64-row halves of the PE array.
  * Both heads' scores for a token chunk land in one 2-bank PSUM quad tile;
    a single ScalarE ACTIVATE [tw, 2*nq] applies exp to the pair (fused
    1/sqrt(64) scale, bf16 out) - ScalarE instruction count matters because
    exp is the attention-phase throughput floor.
  * PV of pair c-1 is emitted BEFORE scores of pair c so ready PE work never
    queues behind score matmuls that are gated on the exp pipeline.
  * Weight/activation DMAs are issued per-chunk, compute-first order, so the
    first K-proj matmul starts ~4us in and phase transitions have no PE gap.
"""

import numpy as np
from contextlib import ExitStack

B, T, NPATCH, APATCH, D, H = 8, 8, 196, 196, 768, 12
HD = D // H                      # 64
SCALE = float(HD) ** -0.5        # 0.125
NT = NPATCH * T                  # 1568 tokens (same count for q and kv side)
P = 128
DC = D // P                      # 6 feature chunks
N_CORES = 8

# token chunks (partition-dim tiling): 12 x 128 + 1 x 32
TOK_CHUNKS = [(i * P, min(P, NT - i * P)) for i in range((NT + P - 1) // P)]
NTC = len(TOK_CHUNKS)            # 13
# nq blocks for the attention/output stage: 4 equal blocks of 392 so the
# last block is not a degenerate latency-bound tail (PSUM quad tiles stay
# 512-padded for bank alignment)
NQB = 512
NQW = NT // 4                    # 392
NQ_BLOCKS = [(s, NQW) for s in range(0, NT, NQW)]
NPAIR = H // 2                   # 6 head pairs

_CACHE: dict = {}
LAST: dict = {"exec_time_ns": None, "trace": None}


def _build_nc(qb_nz: bool, kb_nz: bool, vb_nz: bool, pb_nz: bool):
    import concourse.mybir as mybir
    from concourse import bacc
    from concourse.tile import TileContext

    f32 = mybir.dt.float32
    bf16 = mybir.dt.bfloat16
    AF = mybir.ActivationFunctionType

    nc = bacc.Bacc("TRN2", target_bir_lowering=False, debug=False,
                   num_devices=N_CORES)

    xsT = nc.dram_tensor("xsT", [D, NT], bf16, kind="ExternalInput")
    xaT = nc.dram_tensor("xaT", [D, NT], bf16, kind="ExternalInput")
    qwT = nc.dram_tensor("qwT", [D, D], bf16, kind="ExternalInput")
    kvwT = nc.dram_tensor("kvwT", [D, 2 * D], bf16, kind="ExternalInput")
    projT = nc.dram_tensor("projT", [D, D], bf16, kind="ExternalInput")
    qb = nc.dram_tensor("qb", [P, DC], f32, kind="ExternalInput") if qb_nz else None
    kb = nc.dram_tensor("kb", [P, DC], f32, kind="ExternalInput") if kb_nz else None
    vb = nc.dram_tensor("vb", [1, D], bf16, kind="ExternalInput") if vb_nz else None
    pb = nc.dram_tensor("pb", [1, D], bf16, kind="ExternalInput") if pb_nz else None
    out = nc.dram_tensor("out", [NT, D], f32, kind="ExternalOutput")

    with TileContext(nc) as tc, ExitStack() as ctx:
        consts = ctx.enter_context(tc.tile_pool(name="consts", bufs=1))
        persist = ctx.enter_context(tc.tile_pool(name="persist", bufs=1))

        ones_bf = consts.tile([1, P], bf16, tag="ones_bf")
        nc.gpsimd.memset(ones_bf[:], 1.0)
        qb_sb = kb_sb = vb_sb = pb_sb = None
        if qb_nz:
            qb_sb = consts.tile([P, DC], f32, tag="qb")
            nc.sync.dma_start(qb_sb[:], qb[:])
        if kb_nz:
            kb_sb = consts.tile([P, DC], f32, tag="kb")
            nc.sync.dma_start(kb_sb[:], kb[:])
        if vb_nz:
            vb_sb = consts.tile([1, D], bf16, tag="vb")
            nc.sync.dma_start(vb_sb[:], vb[:])
        if pb_nz:
            pb_sb = consts.tile([1, D], bf16, tag="pb")
            nc.sync.dma_start(pb_sb[:], pb[:])

        # persistent SBUF tensors: K (feature-major) and V (token-major)
        k_feat = [persist.tile([P, NT], bf16, tag=f"k_feat{c}", name=f"k_feat{c}")
                  for c in range(DC)]
        v_st = [persist.tile([P, H, HD + 1], bf16, tag=f"v{i}", name=f"v{i}")
                for i in range(NTC)]

        # phase-2 weights, prefetched during phase 1
        qw_sb = persist.tile([P, DC, D], bf16, tag="qw", name="qw")
        pw_sb = persist.tile([P, DC, D], bf16, tag="pw", name="pw")

        # ---------------- phase 1: K and V projections ----------------
        with ExitStack() as ph:
            wtp = ph.enter_context(tc.tile_pool(name="wtp", bufs=1))
            xfp = ph.enter_context(tc.tile_pool(name="xfp", bufs=1))
            ps1 = ph.enter_context(tc.tile_pool(name="ps1", bufs=6, space="PSUM"))

            kvw_sb = wtp.tile([P, DC, 2 * D], bf16, tag="kvw", name="kvw")
            xs_feat = [xfp.tile([P, NT], bf16, tag=f"xsf{c}", name=f"xsf{c}")
                       for c in range(DC)]
            # compute-first DMA order: K-proj can start after the first
            # kvw/xs chunk pair lands; phase-2 weights stream in behind.
            # The first chunk's transfers are split so the very first
            # matmul group is ready sooner.
            nc.sync.dma_start(kvw_sb[:, 0, :D], kvwT[0:P, :D])
            nc.sync.dma_start(xs_feat[0][:, :NQW], xsT[0:P, :NQW])
            nc.sync.dma_start(kvw_sb[:, 0, D:], kvwT[0:P, D:])
            nc.sync.dma_start(xs_feat[0][:, NQW:], xsT[0:P, NQW:])
            for c in range(1, DC):
                nc.sync.dma_start(kvw_sb[:, c, :], kvwT[c * P:(c + 1) * P, :])
                nc.sync.dma_start(xs_feat[c][:], xsT[c * P:(c + 1) * P, :])
            nc.sync.dma_start(qw_sb[:], qwT.rearrange("(c p) d -> p c d", p=P))
            nc.sync.dma_start(pw_sb[:], projT.rearrange("(c p) d -> p c d", p=P))

            # K projection (feature-major)
            for m in range(DC):
                for (n0, nw) in NQ_BLOCKS:
                    ps = ps1.tile([P, NQB], f32, tag="big", name="kproj")
                    for c in range(DC):
                        nc.tensor.matmul(ps[:, :nw],
                                         kvw_sb[:, c, m * P:(m + 1) * P],
                                         xs_feat[c][:, n0:n0 + nw],
                                         start=(c == 0), stop=(c == DC - 1))
                    if kb_nz:
                        nc.scalar.activation(k_feat[m][:, n0:n0 + nw],
                                             ps[:, :nw], AF.Identity,
                                             bias=kb_sb[:, m:m + 1])
                    else:
                        nc.vector.tensor_copy(k_feat[m][:, n0:n0 + nw],
                                              ps[:, :nw])

            # V projection (token-major, interleaved with ones column)
            for ti, (t0, tw) in enumerate(TOK_CHUNKS):
                for half in range(2):
                    ps = ps1.tile([P, NQB], f32, tag="big", name="vproj")
                    for c in range(DC):
                        nc.tensor.matmul(
                            ps[:tw, :384],
                            xs_feat[c][:, t0:t0 + tw],
                            kvw_sb[:, c, D + half * 384:D + (half + 1) * 384],
                            start=(c == 0), stop=(c == DC - 1 and not vb_nz))
                    if vb_nz:
                        nc.tensor.matmul(
                            ps[:tw, :384], ones_bf[:, :tw],
                            vb_sb[:, half * 384:(half + 1) * 384],
                            start=False, stop=True)
                    nc.vector.tensor_copy(
                        v_st[ti][:tw, half * 6:(half + 1) * 6, :HD],
                        ps[:tw, :384].rearrange("p (h d) -> p h d", d=HD))
                nc.vector.memset(v_st[ti][:tw, :, HD:], 1.0)

        # -------- phase 2: per-block Q proj + attention + O-proj --------
        with ExitStack() as ph:
            xfb = ph.enter_context(tc.tile_pool(name="xfb", bufs=2))
            qfb = ph.enter_context(tc.tile_pool(name="qfb", bufs=2))
            expp = ph.enter_context(tc.tile_pool(name="expp", bufs=4))
            ofp = ph.enter_context(tc.tile_pool(name="ofp", bufs=2))
            otp = ph.enter_context(tc.tile_pool(name="otp", bufs=2))
            nrm = ph.enter_context(tc.tile_pool(name="nrm", bufs=1))
            bcp = ph.enter_context(tc.tile_pool(name="bcp", bufs=3))
            scq = ph.enter_context(tc.tile_pool(name="scq", bufs=2, space="PSUM"))
            pvps = ph.enter_context(tc.tile_pool(name="pvps", bufs=2, space="PSUM"))
            prj = ph.enter_context(tc.tile_pool(name="prj", bufs=2, space="PSUM"))

            def emit_attn_pair(c, blk, nw):
                """Attention for head pair (2c, 2c+1).  The two heads'
                score matmuls go to PE row tiles (0,0)/(64,0) back-to-back
                so they stream concurrently; one ScalarE ACT applies exp to
                the 2-bank quad; PV matmuls trail scores by one token chunk
                so PE and ScalarE both stay continuously fed.  Row 64 of
                each pv (from the V ones-column) is the softmax
                denominator."""
                q_feat = blk["q_feat"]
                pvt = [pvps.tile([HD + 1, NQB], f32, tag="pv", name="pv")
                       for _ in range(2)]

                def pv_chunk(ti, et):
                    t0, tw = TOK_CHUNKS[ti]
                    for par in range(2):
                        nc.tensor.matmul(pvt[par][:, :nw],
                                         v_st[ti][:tw, 2 * c + par, :],
                                         et[:tw, par, :nw],
                                         start=(ti == 0),
                                         stop=(ti == NTC - 1))

                prev_et = None
                for ti, (t0, tw) in enumerate(TOK_CHUNKS):
                    qd = scq.tile([P, 2, NQB], f32, tag="quad", name="squad")
                    for par in range(2):
                        hp = par * HD
                        nc.tensor.matmul(
                            qd[:tw, par, :nw],
                            k_feat[c][hp:hp + HD, t0:t0 + tw],
                            q_feat[c][hp:hp + HD, :nw],
                            start=True, stop=True)
                    et = expp.tile([P, 2, NQB], bf16, tag="exp", name="exp")
                    nc.scalar.activation(et[:tw, :, :nw], qd[:tw, :, :nw],
                                         AF.Exp, scale=SCALE)
                    if prev_et is not None:
                        pv_chunk(ti - 1, prev_et)
                    prev_et = et
                pv_chunk(NTC - 1, prev_et)

                # drain: denominators free-major onto partition 0 (DVE APs
                # need 32-aligned partition bases, so a [12, nq] gather is
                # staged via DMA in norm_stage 0), numerators into out_feat
                # (pre-normalization; scaled in-place next block)
                for par in range(2):
                    h = 2 * c + par
                    nc.vector.tensor_copy(blk["den_st"][0:1, h, :nw],
                                          pvt[par][HD:HD + 1, :nw])
                    nc.vector.tensor_copy(
                        blk["out_feat"][c][par * HD:(par + 1) * HD, :nw],
                        pvt[par][:HD, :nw])

            def norm_stage(blk, stage):
                """Normalize + O-proj of a prior block, split into 6 stages
                interleaved into the successor block's head loop."""
                if blk is None:
                    return
                nw, n0 = blk["nw"], blk["n0"]
                if stage in (0, 1):
                    # partition_broadcast requires dst base partition 0 and
                    # tensor_tensor requires equal input bases: broadcast the
                    # odd head's reciprocal to all 128 partitions, overwrite
                    # partitions 0-63 with the even head's (gpsimd FIFO
                    # orders the writes), then one full-pair mul at base 0.
                    for c in range(3 * stage, 3 * (stage + 1)):
                        bc = bcp.tile([P, NQB], f32, tag="bc", name="bc")
                        nc.gpsimd.partition_broadcast(
                            bc[:, :nw], blk["rec_st"][0:1, 2 * c + 1, :nw])
                        nc.gpsimd.partition_broadcast(
                            bc[:HD, :nw], blk["rec_st"][0:1, 2 * c, :nw])
                        nc.vector.tensor_mul(blk["out_feat"][c][:, :nw],
                                             blk["out_feat"][c][:, :nw],
                                             bc[:, :nw])
                elif stage in (2, 3):
                    # O-projection chunk groups + output DMA
                    chunks = [(cc, min(P, nw - cc)) for cc in range(0, nw, P)]
                    lo = (stage - 2) * 2
                    for (c0, cw) in chunks[lo:lo + 2]:
                        ot = otp.tile([P, D], f32, tag="ot", name="ot")
                        for half in range(2):
                            ps = prj.tile([P, NQB], f32, tag="prj", name="oproj")
                            for c in range(DC):
                                nc.tensor.matmul(
                                    ps[:cw, :384],
                                    blk["out_feat"][c][:, c0:c0 + cw],
                                    pw_sb[:, c, half * 384:(half + 1) * 384],
                                    start=(c == 0),
                                    stop=(c == DC - 1 and not pb_nz))
                            if pb_nz:
                                nc.tensor.matmul(
                                    ps[:cw, :384], ones_bf[:, :cw],
                                    pb_sb[:, half * 384:(half + 1) * 384],
                                    start=False, stop=True)
                            nc.vector.tensor_copy(
                                ot[:cw, half * 384:(half + 1) * 384],
                                ps[:cw, :384])
                        nc.sync.dma_start(out[n0 + c0:n0 + c0 + cw, :],
                                          ot[:cw, :])

            prev = None
            for bi, (n0, nw) in enumerate(NQ_BLOCKS):
                blk = {"n0": n0, "nw": nw}
                # load + project Q for this block (feature-major)
                xa_feat = [xfb.tile([P, NQB], bf16, tag=f"xaf{c}",
                                    name=f"xaf{c}") for c in range(DC)]
                for c in range(DC):
                    nc.sync.dma_start(xa_feat[c][:, :nw],
                                      xaT[c * P:(c + 1) * P, n0:n0 + nw])
                q_feat = [qfb.tile([P, NQB], bf16, tag=f"qf{c}",
                                   name=f"qf{c}") for c in range(DC)]
                for m in range(DC):
                    ps = prj.tile([P, NQB], f32, tag="prj", name="qproj")
                    for c in range(DC):
                        nc.tensor.matmul(ps[:, :nw],
                                         qw_sb[:, c, m * P:(m + 1) * P],
                                         xa_feat[c][:, :nw],
                                         start=(c == 0), stop=(c == DC - 1))
                    if qb_nz:
                        nc.scalar.activation(q_feat[m][:, :nw], ps[:, :nw],
                                             AF.Identity, bias=qb_sb[:, m:m + 1])
                    else:
                        nc.vector.tensor_copy(q_feat[m][:, :nw], ps[:, :nw])
                blk["q_feat"] = q_feat
                blk["out_feat"] = [ofp.tile([P, NQB], bf16, tag=f"of{c}",
                                            name=f"of{c}") for c in range(DC)]
                blk["den_st"] = nrm.tile([1, H, NQB], f32, tag="denst",
                                         name="den_st")
                blk["den12"] = nrm.tile([H, NQB], f32, tag="den", name="den12")
                blk["rec12"] = nrm.tile([H, NQB], f32, tag="rec", name="rec12")
                blk["rec_st"] = nrm.tile([1, H, NQB], f32, tag="recst",
                                         name="rec_st")

                for c in range(NPAIR):
                    emit_attn_pair(c, blk, nw)
                    norm_stage(prev, c)
                # gather the 12 denominators to [12, nq] partitions via DMA
                # (engine APs need 32-aligned partition bases; DMA does not),
                # one batched DVE reciprocal, then scatter back free-major
                # for the gpsimd partition_broadcast reads next block.
                nc.sync.dma_start(blk["den12"][:, :nw],
                                  blk["den_st"][0:1, :, :nw])
                nc.vector.reciprocal(blk["rec12"][:, :nw],
                                     blk["den12"][:, :nw])
                nc.sync.dma_start(blk["rec_st"][0:1, :, :nw],
                                  blk["rec12"][:, :nw])
                prev = blk

            for stage in range(6):
                norm_stage(prev, stage)

    nc.finalize()
    return nc


def kernel(**inputs) -> np.ndarray:
    import ml_dtypes
    bf = ml_dtypes.bfloat16

    s_x = np.asarray(inputs["s_x"], np.float32)
    audio = np.asarray(inputs["audio"], np.float32)
    q_w = np.asarray(inputs["q_w"], np.float32)
    q_b = np.asarray(inputs["q_b"], np.float32)
    kv_w = np.asarray(inputs["kv_w"], np.float32)
    kv_b = np.asarray(inputs["kv_b"], np.float32)
    proj_w = np.asarray(inputs["proj_w"], np.float32)
    proj_b = np.asarray(inputs["proj_b"], np.float32)

    # host prep: layout + O(N*D) positional add + bf16 casts only
    pos_s = (np.asarray(inputs["clip_space_pos"], np.float32)[:, None, :]
             + np.asarray(inputs["clip_temporal_pos"], np.float32)[None, :, :]
             ).reshape(NT, D)
    pos_a = (np.asarray(inputs["audio_space_pos"], np.float32)[:, None, :]
             + np.asarray(inputs["audio_temporal_pos"], np.float32)[None, :, :]
             ).reshape(NT, D)
    qwT = np.ascontiguousarray(q_w.T).astype(bf)
    kvwT = np.ascontiguousarray(kv_w.T).astype(bf)
    projT = np.ascontiguousarray(proj_w.T).astype(bf)
    qb_nz = bool(np.any(q_b))
    kb_nz = bool(np.any(kv_b[:D]))
    vb_nz = bool(np.any(kv_b[D:]))
    pb_nz = bool(np.any(proj_b))

    key = (qb_nz, kb_nz, vb_nz, pb_nz)
    if key not in _CACHE:
        _CACHE[key] = _build_nc(*key)
    nc = _CACHE[key]

    shared = {"qwT": qwT, "kvwT": kvwT, "projT": projT}
    if qb_nz:
        shared["qb"] = np.ascontiguousarray(q_b.reshape(DC, P).T)
    if kb_nz:
        shared["kb"] = np.ascontiguousarray(kv_b[:D].reshape(DC, P).T)
    if vb_nz:
        shared["vb"] = np.ascontiguousarray(kv_b[D:].reshape(1, D)).astype(bf)
    if pb_nz:
        shared["pb"] = np.ascontiguousarray(proj_b.reshape(1, D)).astype(bf)

    in_maps = []
    for b in range(N_CORES):
        m = dict(shared)
        m["xsT"] = np.ascontiguousarray(
            (s_x[1:, b * T:(b + 1) * T, :].reshape(NT, D) + pos_s).T).astype(bf)
        m["xaT"] = np.ascontiguousarray(
            (audio[2:, b * T:(b + 1) * T, :].reshape(NT, D) + pos_a).T).astype(bf)
        in_maps.append(m)

    from concourse.bass_utils import run_bass_kernel_spmd
    res = run_bass_kernel_spmd(nc, in_maps, core_ids=list(range(N_CORES)))
    LAST["exec_time_ns"] = res.exec_time_ns
    LAST["trace"] = res.instructions_and_trace

    out_full = np.empty((2 + APATCH, B * T, D), np.float32)
    out_full[:2] = audio[:2]
    for b in range(N_CORES):
        out_full[2:, b * T:(b + 1) * T, :] = \
            res.results[b]["out"].reshape(APATCH, T, D)
    return out_full


# revision 25
# speedup vs baseline: 1.0178x; 1.0178x over previous
"""Cross-attention (S2Audio) Trainium2 Bass kernel.

Sharding: data-parallel over the clip batch B=8 -> one batch element per
NeuronCore.  Per core the kernel computes, for its batch element b:

  q = (audio_patch + pos_a) @ q_w.T + q_b          (1568, 768)
  k,v = (s_x_patch + pos_s) @ kv_w.T + kv_b        (1568, 768) each
  out = softmax(q k^T / sqrt(64)) v  per 12 heads  -> proj -> (1568, 768)

Host prep is layout/elementwise only: weight transposes, positional-embedding
combine + add (O(N*D)), bf16 casts, sharding slices.  All matmuls/softmax run
on device.

Performance-critical structure (v2):
  * The TRN2 PE clock-gates to 1.2 GHz (HAM K=4/8) whenever it idles; dense
    back-to-back matmul emission keeps it at 2.4 GHz.  All per-head serial
    work (softmax normalization) is OFF the PE queue: denominators come free
    from a ones-column in the PV matmul, reciprocals are batched per block on
    DVE ([12, nq] in one instruction), the partition-broadcast runs on the
    otherwise-idle GpSimd engine, and the final scale is an in-place DVE mul.
    The whole normalize + O-projection of block b-1 is software-pipelined
    into block b's head loop.
  * Scores matmuls have K=64 (head dim) -> 64x128 PE row tiling: the two
    heads of a pair live on SBUF partitions 0-63 / 64-127, their score
    matmuls are emitted interleaved (tile_position (0,0)/(64,0)) so they
    stream CONCURRENTLY through the two 64-row halves of the PE array.
  * Both heads' scores for a token chunk land in one 2-bank PSUM quad tile;
    a single ScalarE ACTIVATE [tw, 2*nq] applies exp to the pair (fused
    1/sqrt(64) scale, bf16 out) - ScalarE instruction count matters because
    exp is the attention-phase throughput floor.
  * PV of pair c-1 is emitted BEFORE scores of pair c so ready PE work never
    queues behind score matmuls that are gated on the exp pipeline.
  * Weight/activation DMAs are issued per-chunk, compute-first order, so the
    first K-proj matmul starts ~4us in and phase transitions have no PE gap.
"""

import numpy as np
from contextlib import ExitStack

B, T, NPATCH, APATCH, D, H = 8, 8, 196, 196, 768, 12
HD = D // H                      # 64
SCALE = float(HD) ** -0.5        # 0.125
NT = NPATCH * T                  # 1568 tokens (same count for q and kv side)
P = 128
DC = D // P                      # 6 feature chunks
N_CORES = 8

# token chunks (partition-dim tiling): 12 x 128 + 1 x 32
TOK_CHUNKS = [(i * P, min(P, NT - i * P)) for i in range((NT + P - 1) // P)]
NTC = len(TOK_CHUNKS)            # 13
# nq blocks for the attention/output stage.  The degenerate 32-query block
# goes FIRST: its latency-bound dependency chains hide under phase 1 (which
# has no ScalarE work), instead of serializing at the end of the kernel.
NQB = 512
NQ_BLOCKS = [(1536, 32), (0, 512), (512, 512), (1024, 512)]
NPAIR = H // 2                   # 6 head pairs

_CACHE: dict = {}
LAST: dict = {"exec_time_ns": None, "trace": None}


def _build_nc(qb_nz: bool, kb_nz: bool, vb_nz: bool, pb_nz: bool):
    import concourse.mybir as mybir
    from concourse import bacc
    from concourse.tile import TileContext

    f32 = mybir.dt.float32
    bf16 = mybir.dt.bfloat16
    AF = mybir.ActivationFunctionType

    nc = bacc.Bacc("TRN2", target_bir_lowering=False, debug=False,
                   num_devices=N_CORES)

    xsT = nc.dram_tensor("xsT", [D, NT], bf16, kind="ExternalInput")
    xaT = nc.dram_tensor("xaT", [D, NT], bf16, kind="ExternalInput")
    qwT = nc.dram_tensor("qwT", [D, D], bf16, kind="ExternalInput")
    kvwT = nc.dram_tensor("kvwT", [D, 2 * D], bf16, kind="ExternalInput")
    projT = nc.dram_tensor("projT", [D, D], bf16, kind="ExternalInput")
    qb = nc.dram_tensor("qb", [P, DC], f32, kind="ExternalInput") if qb_nz else None
    kb = nc.dram_tensor("kb", [P, DC], f32, kind="ExternalInput") if kb_nz else None
    vb = nc.dram_tensor("vb", [1, D], bf16, kind="ExternalInput") if vb_nz else None
    pb = nc.dram_tensor("pb", [1, D], bf16, kind="ExternalInput") if pb_nz else None
    out = nc.dram_tensor("out", [NT, D], f32, kind="ExternalOutput")

    with TileContext(nc) as tc, ExitStack() as ctx:
        consts = ctx.enter_context(tc.tile_pool(name="consts", bufs=1))
        persist = ctx.enter_context(tc.tile_pool(name="persist", bufs=1))

        ones_bf = consts.tile([1, P], bf16, tag="ones_bf")
        nc.gpsimd.memset(ones_bf[:], 1.0)
        qb_sb = kb_sb = vb_sb = pb_sb = None
        if qb_nz:
            qb_sb = consts.tile([P, DC], f32, tag="qb")
            nc.sync.dma_start(qb_sb[:], qb[:])
        if kb_nz:
            kb_sb = consts.tile([P, DC], f32, tag="kb")
            nc.sync.dma_start(kb_sb[:], kb[:])
        if vb_nz:
            vb_sb = consts.tile([1, D], bf16, tag="vb")
            nc.sync.dma_start(vb_sb[:], vb[:])
        if pb_nz:
            pb_sb = consts.tile([1, D], bf16, tag="pb")
            nc.sync.dma_start(pb_sb[:], pb[:])

        # persistent SBUF tensors: K (feature-major) and V (token-major)
        k_feat = [persist.tile([P, NT], bf16, tag=f"k_feat{c}", name=f"k_feat{c}")
                  for c in range(DC)]
        v_st = [persist.tile([P, H, HD + 1], bf16, tag=f"v{i}", name=f"v{i}")
                for i in range(NTC)]

        # phase-2 weights, prefetched during phase 1
        qw_sb = persist.tile([P, DC, D], bf16, tag="qw", name="qw")
        pw_sb = persist.tile([P, DC, D], bf16, tag="pw", name="pw")

        # ---------------- phase 1: K and V projections ----------------
        with ExitStack() as ph:
            wtp = ph.enter_context(tc.tile_pool(name="wtp", bufs=1))
            xfp = ph.enter_context(tc.tile_pool(name="xfp", bufs=1))
            ps1 = ph.enter_context(tc.tile_pool(name="ps1", bufs=6, space="PSUM"))

            kvw_sb = wtp.tile([P, DC, 2 * D], bf16, tag="kvw", name="kvw")
            xs_feat = [xfp.tile([P, NT], bf16, tag=f"xsf{c}", name=f"xsf{c}")
                       for c in range(DC)]
            # compute-first DMA order: K-proj can start after the first
            # kvw/xs chunk pair lands; phase-2 weights stream in behind.
            # The first chunk's transfers are split so the very first
            # matmul group is ready sooner.
            nc.sync.dma_start(kvw_sb[:, 0, :D], kvwT[0:P, :D])
            nc.sync.dma_start(xs_feat[0][:, :NQB], xsT[0:P, :NQB])
            nc.sync.dma_start(kvw_sb[:, 0, D:], kvwT[0:P, D:])
            nc.sync.dma_start(xs_feat[0][:, NQB:], xsT[0:P, NQB:])
            for c in range(1, DC):
                nc.sync.dma_start(kvw_sb[:, c, :], kvwT[c * P:(c + 1) * P, :])
                nc.sync.dma_start(xs_feat[c][:], xsT[c * P:(c + 1) * P, :])
            nc.sync.dma_start(qw_sb[:], qwT.rearrange("(c p) d -> p c d", p=P))
            nc.sync.dma_start(pw_sb[:], projT.rearrange("(c p) d -> p c d", p=P))

            # K projection (feature-major)
            for m in range(DC):
                for (n0, nw) in NQ_BLOCKS:
                    ps = ps1.tile([P, NQB], f32, tag="big", name="kproj")
                    for c in range(DC):
                        nc.tensor.matmul(ps[:, :nw],
                                         kvw_sb[:, c, m * P:(m + 1) * P],
                                         xs_feat[c][:, n0:n0 + nw],
                                         start=(c == 0), stop=(c == DC - 1))
                    if kb_nz:
                        nc.scalar.activation(k_feat[m][:, n0:n0 + nw],
                                             ps[:, :nw], AF.Identity,
                                             bias=kb_sb[:, m:m + 1])
                    else:
                        nc.vector.tensor_copy(k_feat[m][:, n0:n0 + nw],
                                              ps[:, :nw])

            # V projection (token-major, interleaved with ones column)
            for ti, (t0, tw) in enumerate(TOK_CHUNKS):
                for half in range(2):
                    ps = ps1.tile([P, NQB], f32, tag="big", name="vproj")
                    for c in range(DC):
                        nc.tensor.matmul(
                            ps[:tw, :384],
                            xs_feat[c][:, t0:t0 + tw],
                            kvw_sb[:, c, D + half * 384:D + (half + 1) * 384],
                            start=(c == 0), stop=(c == DC - 1 and not vb_nz))
                    if vb_nz:
                        nc.tensor.matmul(
                            ps[:tw, :384], ones_bf[:, :tw],
                            vb_sb[:, half * 384:(half + 1) * 384],
                            start=False, stop=True)
                    nc.vector.tensor_copy(
                        v_st[ti][:tw, half * 6:(half + 1) * 6, :HD],
                        ps[:tw, :384].rearrange("p (h d) -> p h d", d=HD))
                nc.vector.memset(v_st[ti][:tw, :, HD:], 1.0)

        # -------- phase 2: per-block Q proj + attention + O-proj --------
        with ExitStack() as ph:
            xfb = ph.enter_context(tc.tile_pool(name="xfb", bufs=2))
            qfb = ph.enter_context(tc.tile_pool(name="qfb", bufs=2))
            expp = ph.enter_context(tc.tile_pool(name="expp", bufs=4))
            ofp = ph.enter_context(tc.tile_pool(name="ofp", bufs=2))
            otp = ph.enter_context(tc.tile_pool(name="otp", bufs=2))
            nrm = ph.enter_context(tc.tile_pool(name="nrm", bufs=1))
            bcp = ph.enter_context(tc.tile_pool(name="bcp", bufs=3))
            scq = ph.enter_context(tc.tile_pool(name="scq", bufs=2, space="PSUM"))
            pvps = ph.enter_context(tc.tile_pool(name="pvps", bufs=2, space="PSUM"))
            prj = ph.enter_context(tc.tile_pool(name="prj", bufs=2, space="PSUM"))

            def emit_attn_pair(c, blk, nw):
                """Attention for head pair (2c, 2c+1).  The two heads'
                score matmuls go to PE row tiles (0,0)/(64,0) back-to-back
                so they stream concurrently; one ScalarE ACT applies exp to
                the 2-bank quad; PV matmuls trail scores by one token chunk
                so PE and ScalarE both stay continuously fed.  Row 64 of
                each pv (from the V ones-column) is the softmax
                denominator."""
                q_feat = blk["q_feat"]
                pvt = [pvps.tile([HD + 1, NQB], f32, tag="pv", name="pv")
                       for _ in range(2)]

                def pv_chunk(ti, et):
                    t0, tw = TOK_CHUNKS[ti]
                    for par in range(2):
                        nc.tensor.matmul(pvt[par][:, :nw],
                                         v_st[ti][:tw, 2 * c + par, :],
                                         et[:tw, par, :nw],
                                         start=(ti == 0),
                                         stop=(ti == NTC - 1))

                prev_et = None
                for ti, (t0, tw) in enumerate(TOK_CHUNKS):
                    qd = scq.tile([P, 2, NQB], f32, tag="quad", name="squad")
                    for par in range(2):
                        hp = par * HD
                        nc.tensor.matmul(
                            qd[:tw, par, :nw],
                            k_feat[c][hp:hp + HD, t0:t0 + tw],
                            q_feat[c][hp:hp + HD, :nw],
                            start=True, stop=True)
                    et = expp.tile([P, 2, NQB], bf16, tag="exp", name="exp")
                    nc.scalar.activation(et[:tw, :, :nw], qd[:tw, :, :nw],
                                         AF.Exp, scale=SCALE)
                    if prev_et is not None:
                        pv_chunk(ti - 1, prev_et)
                    prev_et = et
                pv_chunk(NTC - 1, prev_et)

                # drain: denominators free-major onto partition 0 (DVE APs
                # need 32-aligned partition bases, so a [12, nq] gather is
                # staged via DMA in norm_stage 0), numerators into out_feat
                # (pre-normalization; scaled in-place next block)
                for par in range(2):
                    h = 2 * c + par
                    nc.vector.tensor_copy(blk["den_st"][0:1, h, :nw],
                                          pvt[par][HD:HD + 1, :nw])
                    nc.vector.tensor_copy(
                        blk["out_feat"][c][par * HD:(par + 1) * HD, :nw],
                        pvt[par][:HD, :nw])

            def norm_stage(blk, stage):
                """Normalize + O-proj of a prior block, split into 6 stages
                interleaved into the successor block's head loop."""
                if blk is None:
                    return
                nw, n0 = blk["nw"], blk["n0"]
                if stage in (0, 1):
                    # partition_broadcast requires dst base partition 0 and
                    # tensor_tensor requires equal input bases: broadcast the
                    # odd head's reciprocal to all 128 partitions, overwrite
                    # partitions 0-63 with the even head's (gpsimd FIFO
                    # orders the writes), then one full-pair mul at base 0.
                    for c in range(3 * stage, 3 * (stage + 1)):
                        bc = bcp.tile([P, NQB], f32, tag="bc", name="bc")
                        nc.gpsimd.partition_broadcast(
                            bc[:, :nw], blk["rec_st"][0:1, 2 * c + 1, :nw])
                        nc.gpsimd.partition_broadcast(
                            bc[:HD, :nw], blk["rec_st"][0:1, 2 * c, :nw])
                        nc.vector.tensor_mul(blk["out_feat"][c][:, :nw],
                                             blk["out_feat"][c][:, :nw],
                                             bc[:, :nw])
                elif stage in (2, 3):
                    # O-projection chunk groups + output DMA
                    chunks = [(cc, min(P, nw - cc)) for cc in range(0, nw, P)]
                    lo = (stage - 2) * 2
                    for (c0, cw) in chunks[lo:lo + 2]:
                        ot = otp.tile([P, D], f32, tag="ot", name="ot")
                        for half in range(2):
                            ps = prj.tile([P, NQB], f32, tag="prj", name="oproj")
                            for c in range(DC):
                                nc.tensor.matmul(
                                    ps[:cw, :384],
                                    blk["out_feat"][c][:, c0:c0 + cw],
                                    pw_sb[:, c, half * 384:(half + 1) * 384],
                                    start=(c == 0),
                                    stop=(c == DC - 1 and not pb_nz))
                            if pb_nz:
                                nc.tensor.matmul(
                                    ps[:cw, :384], ones_bf[:, :cw],
                                    pb_sb[:, half * 384:(half + 1) * 384],
                                    start=False, stop=True)
                            nc.vector.tensor_copy(
                                ot[:cw, half * 384:(half + 1) * 384],
                                ps[:cw, :384])
                        nc.sync.dma_start(out[n0 + c0:n0 + c0 + cw, :],
                                          ot[:cw, :])

            prev = None
            for bi, (n0, nw) in enumerate(NQ_BLOCKS):
                blk = {"n0": n0, "nw": nw}
                is_last = bi == len(NQ_BLOCKS) - 1
                # load + project Q for this block (feature-major)
                xa_feat = [xfb.tile([P, NQB], bf16, tag=f"xaf{c}",
                                    name=f"xaf{c}") for c in range(DC)]
                for c in range(DC):
                    nc.sync.dma_start(xa_feat[c][:, :nw],
                                      xaT[c * P:(c + 1) * P, n0:n0 + nw])
                q_feat = [qfb.tile([P, NQB], bf16, tag=f"qf{c}",
                                   name=f"qf{c}") for c in range(DC)]
                for m in range(DC):
                    ps = prj.tile([P, NQB], f32, tag="prj", name="qproj")
                    for c in range(DC):
                        nc.tensor.matmul(ps[:, :nw],
                                         qw_sb[:, c, m * P:(m + 1) * P],
                                         xa_feat[c][:, :nw],
                                         start=(c == 0), stop=(c == DC - 1))
                    if qb_nz:
                        nc.scalar.activation(q_feat[m][:, :nw], ps[:, :nw],
                                             AF.Identity, bias=qb_sb[:, m:m + 1])
                    else:
                        nc.vector.tensor_copy(q_feat[m][:, :nw], ps[:, :nw])
                blk["q_feat"] = q_feat
                blk["out_feat"] = [ofp.tile([P, NQB], bf16, tag=f"of{c}",
                                            name=f"of{c}") for c in range(DC)]
                blk["den_st"] = nrm.tile([1, H, NQB], f32, tag="denst",
                                         name="den_st")
                blk["den12"] = nrm.tile([H, NQB], f32, tag="den", name="den12")
                blk["rec12"] = nrm.tile([H, NQB], f32, tag="rec", name="rec12")
                blk["rec_st"] = nrm.tile([1, H, NQB], f32, tag="recst",
                                         name="rec_st", bufs=2)

                for c in range(NPAIR):
                    emit_attn_pair(c, blk, nw)
                    norm_stage(prev, c)
                    if is_last:
                        # final block: normalize per pair inline so the
                        # end-of-kernel serial chain is one pair deep, not
                        # a whole block.  Same DMA round-trip as below but
                        # on a [2, nq] slice (32-aligned bases via pair
                        # staging tiles at partition 0).
                        dp = nrm.tile([2, NQB], f32, tag="dpair",
                                      name="dpair", bufs=2)
                        rp = nrm.tile([2, NQB], f32, tag="rpair",
                                      name="rpair", bufs=2)
                        nc.sync.dma_start(
                            dp[:, :nw],
                            blk["den_st"][0:1, 2 * c:2 * c + 2, :nw])
                        nc.vector.reciprocal(rp[:, :nw], dp[:, :nw])
                        nc.sync.dma_start(
                            blk["rec_st"][0:1, 2 * c:2 * c + 2, :nw],
                            rp[:, :nw])
                        bc = bcp.tile([P, NQB], f32, tag="bc", name="bc")
                        nc.gpsimd.partition_broadcast(
                            bc[:, :nw], blk["rec_st"][0:1, 2 * c + 1, :nw])
                        nc.gpsimd.partition_broadcast(
                            bc[:HD, :nw], blk["rec_st"][0:1, 2 * c, :nw])
                        nc.vector.tensor_mul(blk["out_feat"][c][:, :nw],
                                             blk["out_feat"][c][:, :nw],
                                             bc[:, :nw])
                if is_last:
                    norm_stage(blk, 2)
                    norm_stage(blk, 3)
                else:
                    # gather the 12 denominators to [12, nq] partitions via
                    # DMA (engine APs need 32-aligned partition bases; DMA
                    # does not), one batched DVE reciprocal, then scatter
                    # back free-major for the gpsimd partition_broadcast
                    # reads next block.
                    nc.sync.dma_start(blk["den12"][:, :nw],
                                      blk["den_st"][0:1, :, :nw])
                    nc.vector.reciprocal(blk["rec12"][:, :nw],
                                         blk["den12"][:, :nw])
                    nc.sync.dma_start(blk["rec_st"][0:1, :, :nw],
                                      blk["rec12"][:, :nw])
                prev = blk

    nc.finalize()
    return nc


def kernel(**inputs) -> np.ndarray:
    import ml_dtypes
    bf = ml_dtypes.bfloat16

    s_x = np.asarray(inputs["s_x"], np.float32)
    audio = np.asarray(inputs["audio"], np.float32)
    q_w = np.asarray(inputs["q_w"], np.float32)
    q_b = np.asarray(inputs["q_b"], np.float32)
    kv_w = np.asarray(inputs["kv_w"], np.float32)
    kv_b = np.asarray(inputs["kv_b"], np.float32)
    proj_w = np.asarray(inputs["proj_w"], np.float32)
    proj_b = np.asarray(inputs["proj_b"], np.float32)

    # host prep: layout + O(N*D) positional add + bf16 casts only
    pos_s = (np.asarray(inputs["clip_space_pos"], np.float32)[:, None, :]
             + np.asarray(inputs["clip_temporal_pos"], np.float32)[None, :, :]
             ).reshape(NT, D)
    pos_a = (np.asarray(inputs["audio_space_pos"], np.float32)[:, None, :]
             + np.asarray(inputs["audio_temporal_pos"], np.float32)[None, :, :]
             ).reshape(NT, D)
    qwT = np.ascontiguousarray(q_w.T).astype(bf)
    kvwT = np.ascontiguousarray(kv_w.T).astype(bf)
    projT = np.ascontiguousarray(proj_w.T).astype(bf)
    qb_nz = bool(np.any(q_b))
    kb_nz = bool(np.any(kv_b[:D]))
    vb_nz = bool(np.any(kv_b[D:]))
    pb_nz = bool(np.any(proj_b))

    key = (qb_nz, kb_nz, vb_nz, pb_nz)
    if key not in _CACHE:
        _CACHE[key] = _build_nc(*key)
    nc = _CACHE[key]

    shared = {"qwT": qwT, "kvwT": kvwT, "projT": projT}
    if qb_nz:
        shared["qb"] = np.ascontiguousarray(q_b.reshape(DC, P).T)
    if kb_nz:
        shared["kb"] = np.ascontiguousarray(kv_b[:D].reshape(DC, P).T)
    if vb_nz:
        shared["vb"] = np.ascontiguousarray(kv_b[D:].reshape(1, D)).astype(bf)
    if pb_nz:
        shared["pb"] = np.ascontiguousarray(proj_b.reshape(1, D)).astype(bf)

    in_maps = []
    for b in range(N_CORES):
        m = dict(shared)
        m["xsT"] = np.ascontiguousarray(
            (s_x[1:, b * T:(b + 1) * T, :].reshape(NT, D) + pos_s).T).astype(bf)
        m["xaT"] = np.ascontiguousarray(
            (audio[2:, b * T:(b + 1) * T, :].reshape(NT, D) + pos_a).T).astype(bf)
        in_maps.append(m)

    from concourse.bass_utils import run_bass_kernel_spmd
    res = run_bass_kernel_spmd(nc, in_maps, core_ids=list(range(N_CORES)))
    LAST["exec_time_ns"] = res.exec_time_ns
    LAST["trace"] = res.instructions_and_trace

    out_full = np.empty((2 + APATCH, B * T, D), np.float32)
    out_full[:2] = audio[:2]
    for b in range(N_CORES):
        out_full[2:, b * T:(b + 1) * T, :] = \
            res.results[b]["out"].reshape(APATCH, T, D)
    return out_full


# revision 28
# speedup vs baseline: 1.0494x; 1.0311x over previous
"""Cross-attention (S2Audio) Trainium2 Bass kernel.

Sharding: data-parallel over the clip batch B=8 -> one batch element per
NeuronCore.  Per core the kernel computes, for its batch element b:

  q = (audio_patch + pos_a) @ q_w.T + q_b          (1568, 768)
  k,v = (s_x_patch + pos_s) @ kv_w.T + kv_b        (1568, 768) each
  out = softmax(q k^T / sqrt(64)) v  per 12 heads  -> proj -> (1568, 768)

Host prep is layout/elementwise only: weight transposes, positional-embedding
combine + add (O(N*D)), bf16 casts, sharding slices.  All matmuls/softmax run
on device.

Performance-critical structure (v2):
  * The TRN2 PE clock-gates to 1.2 GHz (HAM K=4/8) whenever it idles; dense
    back-to-back matmul emission keeps it at 2.4 GHz.  All per-head serial
    work (softmax normalization) is OFF the PE queue: denominators come free
    from a ones-column in the PV matmul, reciprocals are batched per block on
    DVE ([12, nq] in one instruction), the partition-broadcast runs on the
    otherwise-idle GpSimd engine, and the final scale is an in-place DVE mul.
    The whole normalize + O-projection of block b-1 is software-pipelined
    into block b's head loop.
  * Scores matmuls have K=64 (head dim) -> 64x128 PE row tiling: the two
    heads of a pair live on SBUF partitions 0-63 / 64-127, their score
    matmuls are emitted interleaved (tile_position (0,0)/(64,0)) so they
    stream CONCURRENTLY through the two 64-row halves of the PE array.
  * Both heads' scores for a token chunk land in one 2-bank PSUM quad tile;
    a single ScalarE ACTIVATE [tw, 2*nq] applies exp to the pair (fused
    1/sqrt(64) scale, bf16 out) - ScalarE instruction count matters because
    exp is the attention-phase throughput floor.
  * PV of pair c-1 is emitted BEFORE scores of pair c so ready PE work never
    queues behind score matmuls that are gated on the exp pipeline.
  * Weight/activation DMAs are issued per-chunk, compute-first order, so the
    first K-proj matmul starts ~4us in and phase transitions have no PE gap.
"""

import numpy as np
from contextlib import ExitStack

B, T, NPATCH, APATCH, D, H = 8, 8, 196, 196, 768, 12
HD = D // H                      # 64
SCALE = float(HD) ** -0.5        # 0.125
NT = NPATCH * T                  # 1568 tokens (same count for q and kv side)
P = 128
DC = D // P                      # 6 feature chunks
N_CORES = 8

# token chunks (partition-dim tiling): 12 x 128 + 1 x 32
TOK_CHUNKS = [(i * P, min(P, NT - i * P)) for i in range((NT + P - 1) // P)]
NTC = len(TOK_CHUNKS)            # 13
# nq blocks for the attention/output stage
NQB = 512
NQ_BLOCKS = [(s, min(NQB, NT - s)) for s in range(0, NT, NQB)]
NPAIR = H // 2                   # 6 head pairs

_CACHE: dict = {}
LAST: dict = {"exec_time_ns": None, "trace": None}


def _build_nc(qb_nz: bool, kb_nz: bool, vb_nz: bool, pb_nz: bool):
    import concourse.mybir as mybir
    from concourse import bacc
    from concourse.tile import TileContext

    f32 = mybir.dt.float32
    bf16 = mybir.dt.bfloat16
    AF = mybir.ActivationFunctionType

    nc = bacc.Bacc("TRN2", target_bir_lowering=False, debug=False,
                   num_devices=N_CORES)

    xsT = nc.dram_tensor("xsT", [D, NT], bf16, kind="ExternalInput")
    xaT = nc.dram_tensor("xaT", [D, NT], bf16, kind="ExternalInput")
    qwT = nc.dram_tensor("qwT", [D, D], bf16, kind="ExternalInput")
    kvwT = nc.dram_tensor("kvwT", [D, 2 * D], bf16, kind="ExternalInput")
    projT = nc.dram_tensor("projT", [D, D], bf16, kind="ExternalInput")
    qb = nc.dram_tensor("qb", [P, DC], f32, kind="ExternalInput") if qb_nz else None
    kb = nc.dram_tensor("kb", [P, DC], f32, kind="ExternalInput") if kb_nz else None
    vb = nc.dram_tensor("vb", [1, D], bf16, kind="ExternalInput") if vb_nz else None
    pb = nc.dram_tensor("pb", [1, D], bf16, kind="ExternalInput") if pb_nz else None
    out = nc.dram_tensor("out", [NT, D], f32, kind="ExternalOutput")

    with TileContext(nc) as tc, ExitStack() as ctx:
        consts = ctx.enter_context(tc.tile_pool(name="consts", bufs=1))
        persist = ctx.enter_context(tc.tile_pool(name="persist", bufs=1))

        ones_bf = consts.tile([1, P], bf16, tag="ones_bf")
        nc.gpsimd.memset(ones_bf[:], 1.0)
        qb_sb = kb_sb = vb_sb = pb_sb = None
        if qb_nz:
            qb_sb = consts.tile([P, DC], f32, tag="qb")
            nc.sync.dma_start(qb_sb[:], qb[:])
        if kb_nz:
            kb_sb = consts.tile([P, DC], f32, tag="kb")
            nc.sync.dma_start(kb_sb[:], kb[:])
        if vb_nz:
            vb_sb = consts.tile([1, D], bf16, tag="vb")
            nc.sync.dma_start(vb_sb[:], vb[:])
        if pb_nz:
            pb_sb = consts.tile([1, D], bf16, tag="pb")
            nc.sync.dma_start(pb_sb[:], pb[:])

        # persistent SBUF tensors: K (feature-major) and V (token-major)
        k_feat = [persist.tile([P, NT], bf16, tag=f"k_feat{c}", name=f"k_feat{c}")
                  for c in range(DC)]
        v_st = [persist.tile([P, H, HD + 1], bf16, tag=f"v{i}", name=f"v{i}")
                for i in range(NTC)]

        # phase-2 weights, prefetched during phase 1
        qw_sb = persist.tile([P, DC, D], bf16, tag="qw", name="qw")
        pw_sb = persist.tile([P, DC, D], bf16, tag="pw", name="pw")

        # ---------------- phase 1: K and V projections ----------------
        with ExitStack() as ph:
            wtp = ph.enter_context(tc.tile_pool(name="wtp", bufs=1))
            xfp = ph.enter_context(tc.tile_pool(name="xfp", bufs=1))
            ps1 = ph.enter_context(tc.tile_pool(name="ps1", bufs=6, space="PSUM"))

            kvw_sb = wtp.tile([P, DC, 2 * D], bf16, tag="kvw", name="kvw")
            xs_feat = [xfp.tile([P, NT], bf16, tag=f"xsf{c}", name=f"xsf{c}")
                       for c in range(DC)]
            # compute-first DMA order: K-proj can start after the first
            # kvw/xs chunk pair lands; phase-2 weights stream in behind.
            # The first chunk's transfers are split so the very first
            # matmul group is ready sooner.
            # spread the startup loads over three hardware DMA queues
            # (sync / scalar / gpsimd triggers) so the first matmul and the
            # first exp are not gated on one serial queue.
            nc.sync.dma_start(kvw_sb[:, 0, :D], kvwT[0:P, :D])
            nc.sync.dma_start(xs_feat[0][:, :NQB], xsT[0:P, :NQB])
            nc.sync.dma_start(kvw_sb[:, 0, D:], kvwT[0:P, D:])
            nc.sync.dma_start(xs_feat[0][:, NQB:], xsT[0:P, NQB:])
            for c in range(1, DC):
                q = nc.sync if c % 2 else nc.gpsimd
                q.dma_start(kvw_sb[:, c, :], kvwT[c * P:(c + 1) * P, :])
                q.dma_start(xs_feat[c][:], xsT[c * P:(c + 1) * P, :])
            nc.scalar.dma_start(qw_sb[:],
                                qwT.rearrange("(c p) d -> p c d", p=P))
            nc.scalar.dma_start(pw_sb[:],
                                projT.rearrange("(c p) d -> p c d", p=P))

            # K projection (feature-major)
            for m in range(DC):
                for (n0, nw) in NQ_BLOCKS:
                    ps = ps1.tile([P, NQB], f32, tag="big", name="kproj")
                    for c in range(DC):
                        nc.tensor.matmul(ps[:, :nw],
                                         kvw_sb[:, c, m * P:(m + 1) * P],
                                         xs_feat[c][:, n0:n0 + nw],
                                         start=(c == 0), stop=(c == DC - 1))
                    if kb_nz:
                        nc.scalar.activation(k_feat[m][:, n0:n0 + nw],
                                             ps[:, :nw], AF.Identity,
                                             bias=kb_sb[:, m:m + 1])
                    else:
                        nc.vector.tensor_copy(k_feat[m][:, n0:n0 + nw],
                                              ps[:, :nw])

            # V projection (token-major, interleaved with ones column)
            for ti, (t0, tw) in enumerate(TOK_CHUNKS):
                for half in range(2):
                    ps = ps1.tile([P, NQB], f32, tag="big", name="vproj")
                    for c in range(DC):
                        nc.tensor.matmul(
                            ps[:tw, :384],
                            xs_feat[c][:, t0:t0 + tw],
                            kvw_sb[:, c, D + half * 384:D + (half + 1) * 384],
                            start=(c == 0), stop=(c == DC - 1 and not vb_nz))
                    if vb_nz:
                        nc.tensor.matmul(
                            ps[:tw, :384], ones_bf[:, :tw],
                            vb_sb[:, half * 384:(half + 1) * 384],
                            start=False, stop=True)
                    nc.vector.tensor_copy(
                        v_st[ti][:tw, half * 6:(half + 1) * 6, :HD],
                        ps[:tw, :384].rearrange("p (h d) -> p h d", d=HD))
                nc.vector.memset(v_st[ti][:tw, :, HD:], 1.0)

        # -------- phase 2: per-block Q proj + attention + O-proj --------
        with ExitStack() as ph:
            xfb = ph.enter_context(tc.tile_pool(name="xfb", bufs=2))
            qfb = ph.enter_context(tc.tile_pool(name="qfb", bufs=2))
            expp = ph.enter_context(tc.tile_pool(name="expp", bufs=4))
            ofp = ph.enter_context(tc.tile_pool(name="ofp", bufs=2))
            otp = ph.enter_context(tc.tile_pool(name="otp", bufs=2))
            nrm = ph.enter_context(tc.tile_pool(name="nrm", bufs=1))
            bcp = ph.enter_context(tc.tile_pool(name="bcp", bufs=3))
            scq = ph.enter_context(tc.tile_pool(name="scq", bufs=2, space="PSUM"))
            pvps = ph.enter_context(tc.tile_pool(name="pvps", bufs=2, space="PSUM"))
            prj = ph.enter_context(tc.tile_pool(name="prj", bufs=2, space="PSUM"))

            def emit_attn_pair(c, blk, nw):
                """Attention for head pair (2c, 2c+1).  The two heads'
                score matmuls go to PE row tiles (0,0)/(64,0) back-to-back
                so they stream concurrently; one ScalarE ACT applies exp to
                the 2-bank quad; PV matmuls trail scores by one token chunk
                so PE and ScalarE both stay continuously fed.  Row 64 of
                each pv (from the V ones-column) is the softmax
                denominator."""
                q_feat = blk["q_feat"]
                pvt = [pvps.tile([HD + 1, NQB], f32, tag="pv", name="pv")
                       for _ in range(2)]

                def pv_chunk(ti, et):
                    t0, tw = TOK_CHUNKS[ti]
                    for par in range(2):
                        nc.tensor.matmul(pvt[par][:, :nw],
                                         v_st[ti][:tw, 2 * c + par, :],
                                         et[:tw, par, :nw],
                                         start=(ti == 0),
                                         stop=(ti == NTC - 1))

                if nw * NTC <= NQB:
                    # narrow tail block: pack all 13 chunks' scores
                    # side-by-side in one 2-bank quad so the pair needs a
                    # single ACT instead of 13 tiny ones (rows >= tw of a
                    # slot are unwritten garbage; never read by PV).
                    qd = scq.tile([P, 2, NQB], f32, tag="quad", name="squad")
                    for ti, (t0, tw) in enumerate(TOK_CHUNKS):
                        for par in range(2):
                            hp = par * HD
                            nc.tensor.matmul(
                                qd[:tw, par, ti * nw:(ti + 1) * nw],
                                k_feat[c][hp:hp + HD, t0:t0 + tw],
                                q_feat[c][hp:hp + HD, :nw],
                                start=True, stop=True)
                    et = expp.tile([P, 2, NQB], bf16, tag="exp", name="exp")
                    nc.scalar.activation(et[:, :, :NTC * nw],
                                         qd[:, :, :NTC * nw],
                                         AF.Exp, scale=SCALE)
                    for ti, (t0, tw) in enumerate(TOK_CHUNKS):
                        for par in range(2):
                            nc.tensor.matmul(
                                pvt[par][:, :nw],
                                v_st[ti][:tw, 2 * c + par, :],
                                et[:tw, par, ti * nw:(ti + 1) * nw],
                                start=(ti == 0), stop=(ti == NTC - 1))
                else:
                    prev_et = None
                    for ti, (t0, tw) in enumerate(TOK_CHUNKS):
                        qd = scq.tile([P, 2, NQB], f32, tag="quad",
                                      name="squad")
                        for par in range(2):
                            hp = par * HD
                            nc.tensor.matmul(
                                qd[:tw, par, :nw],
                                k_feat[c][hp:hp + HD, t0:t0 + tw],
                                q_feat[c][hp:hp + HD, :nw],
                                start=True, stop=True)
                        et = expp.tile([P, 2, NQB], bf16, tag="exp",
                                       name="exp")
                        nc.scalar.activation(et[:tw, :, :nw],
                                             qd[:tw, :, :nw],
                                             AF.Exp, scale=SCALE)
                        if prev_et is not None:
                            pv_chunk(ti - 1, prev_et)
                        prev_et = et
                    pv_chunk(NTC - 1, prev_et)

                # drain: denominators free-major onto partition 0 (DVE APs
                # need 32-aligned partition bases, so a [12, nq] gather is
                # staged via DMA in norm_stage 0), numerators into out_feat
                # (pre-normalization; scaled in-place next block)
                for par in range(2):
                    h = 2 * c + par
                    nc.vector.tensor_copy(blk["den_st"][0:1, h, :nw],
                                          pvt[par][HD:HD + 1, :nw])
                    nc.vector.tensor_copy(
                        blk["out_feat"][c][par * HD:(par + 1) * HD, :nw],
                        pvt[par][:HD, :nw])

            def norm_stage(blk, stage):
                """Normalize + O-proj of a prior block, split into 6 stages
                interleaved into the successor block's head loop."""
                if blk is None:
                    return
                nw, n0 = blk["nw"], blk["n0"]
                if stage in (0, 1):
                    # partition_broadcast requires dst base partition 0 and
                    # tensor_tensor requires equal input bases: broadcast the
                    # odd head's reciprocal to all 128 partitions, overwrite
                    # partitions 0-63 with the even head's (gpsimd FIFO
                    # orders the writes), then one full-pair mul at base 0.
                    for c in range(3 * stage, 3 * (stage + 1)):
                        bc = bcp.tile([P, NQB], f32, tag="bc", name="bc")
                        nc.gpsimd.partition_broadcast(
                            bc[:, :nw], blk["rec_st"][0:1, 2 * c + 1, :nw])
                        nc.gpsimd.partition_broadcast(
                            bc[:HD, :nw], blk["rec_st"][0:1, 2 * c, :nw])
                        nc.vector.tensor_mul(blk["out_feat"][c][:, :nw],
                                             blk["out_feat"][c][:, :nw],
                                             bc[:, :nw])
                elif stage in (2, 3):
                    # O-projection chunk groups + output DMA
                    chunks = [(cc, min(P, nw - cc)) for cc in range(0, nw, P)]
                    lo = (stage - 2) * 2
                    for (c0, cw) in chunks[lo:lo + 2]:
                        ot = otp.tile([P, D], f32, tag="ot", name="ot")
                        for half in range(2):
                            ps = prj.tile([P, NQB], f32, tag="prj", name="oproj")
                            for c in range(DC):
                                nc.tensor.matmul(
                                    ps[:cw, :384],
                                    blk["out_feat"][c][:, c0:c0 + cw],
                                    pw_sb[:, c, half * 384:(half + 1) * 384],
                                    start=(c == 0),
                                    stop=(c == DC - 1 and not pb_nz))
                            if pb_nz:
                                nc.tensor.matmul(
                                    ps[:cw, :384], ones_bf[:, :cw],
                                    pb_sb[:, half * 384:(half + 1) * 384],
                                    start=False, stop=True)
                            nc.vector.tensor_copy(
                                ot[:cw, half * 384:(half + 1) * 384],
                                ps[:cw, :384])
                        nc.sync.dma_start(out[n0 + c0:n0 + c0 + cw, :],
                                          ot[:cw, :])

            prev = None
            for bi, (n0, nw) in enumerate(NQ_BLOCKS):
                blk = {"n0": n0, "nw": nw}
                is_last = bi == len(NQ_BLOCKS) - 1
                # load + project Q for this block (feature-major)
                xa_feat = [xfb.tile([P, NQB], bf16, tag=f"xaf{c}",
                                    name=f"xaf{c}") for c in range(DC)]
                for c in range(DC):
                    nc.sync.dma_start(xa_feat[c][:, :nw],
                                      xaT[c * P:(c + 1) * P, n0:n0 + nw])
                q_feat = [qfb.tile([P, NQB], bf16, tag=f"qf{c}",
                                   name=f"qf{c}") for c in range(DC)]
                for m in range(DC):
                    ps = prj.tile([P, NQB], f32, tag="prj", name="qproj")
                    for c in range(DC):
                        nc.tensor.matmul(ps[:, :nw],
                                         qw_sb[:, c, m * P:(m + 1) * P],
                                         xa_feat[c][:, :nw],
                                         start=(c == 0), stop=(c == DC - 1))
                    if qb_nz:
                        nc.scalar.activation(q_feat[m][:, :nw], ps[:, :nw],
                                             AF.Identity, bias=qb_sb[:, m:m + 1])
                    else:
                        nc.vector.tensor_copy(q_feat[m][:, :nw], ps[:, :nw])
                blk["q_feat"] = q_feat
                blk["out_feat"] = [ofp.tile([P, NQB], bf16, tag=f"of{c}",
                                            name=f"of{c}") for c in range(DC)]
                blk["den_st"] = nrm.tile([1, H, NQB], f32, tag="denst",
                                         name="den_st")
                blk["den12"] = nrm.tile([H, NQB], f32, tag="den", name="den12")
                blk["rec12"] = nrm.tile([H, NQB], f32, tag="rec", name="rec12")
                blk["rec_st"] = nrm.tile([1, H, NQB], f32, tag="recst",
                                         name="rec_st", bufs=2)

                for c in range(NPAIR):
                    emit_attn_pair(c, blk, nw)
                    norm_stage(prev, c)
                    if is_last:
                        # final block: normalize per pair inline so the
                        # end-of-kernel serial chain is one pair deep, not
                        # a whole block.  Same DMA round-trip as below but
                        # on a [2, nq] slice (32-aligned bases via pair
                        # staging tiles at partition 0).
                        dp = nrm.tile([2, NQB], f32, tag="dpair",
                                      name="dpair", bufs=2)
                        rp = nrm.tile([2, NQB], f32, tag="rpair",
                                      name="rpair", bufs=2)
                        nc.sync.dma_start(
                            dp[:, :nw],
                            blk["den_st"][0:1, 2 * c:2 * c + 2, :nw])
                        nc.vector.reciprocal(rp[:, :nw], dp[:, :nw])
                        nc.sync.dma_start(
                            blk["rec_st"][0:1, 2 * c:2 * c + 2, :nw],
                            rp[:, :nw])
                        bc = bcp.tile([P, NQB], f32, tag="bc", name="bc")
                        nc.gpsimd.partition_broadcast(
                            bc[:, :nw], blk["rec_st"][0:1, 2 * c + 1, :nw])
                        nc.gpsimd.partition_broadcast(
                            bc[:HD, :nw], blk["rec_st"][0:1, 2 * c, :nw])
                        nc.vector.tensor_mul(blk["out_feat"][c][:, :nw],
                                             blk["out_feat"][c][:, :nw],
                                             bc[:, :nw])
                if is_last:
                    norm_stage(blk, 2)
                    norm_stage(blk, 3)
                else:
                    # gather the 12 denominators to [12, nq] partitions via
                    # DMA (engine APs need 32-aligned partition bases; DMA
                    # does not), one batched DVE reciprocal, then scatter
                    # back free-major for the gpsimd partition_broadcast
                    # reads next block.
                    nc.sync.dma_start(blk["den12"][:, :nw],
                                      blk["den_st"][0:1, :, :nw])
                    nc.vector.reciprocal(blk["rec12"][:, :nw],
                                         blk["den12"][:, :nw])
                    nc.sync.dma_start(blk["rec_st"][0:1, :, :nw],
                                      blk["rec12"][:, :nw])
                prev = blk

    nc.finalize()
    return nc


def kernel(**inputs) -> np.ndarray:
    import ml_dtypes
    bf = ml_dtypes.bfloat16

    s_x = np.asarray(inputs["s_x"], np.float32)
    audio = np.asarray(inputs["audio"], np.float32)
    q_w = np.asarray(inputs["q_w"], np.float32)
    q_b = np.asarray(inputs["q_b"], np.float32)
    kv_w = np.asarray(inputs["kv_w"], np.float32)
    kv_b = np.asarray(inputs["kv_b"], np.float32)
    proj_w = np.asarray(inputs["proj_w"], np.float32)
    proj_b = np.asarray(inputs["proj_b"], np.float32)

    # host prep: layout + O(N*D) positional add + bf16 casts only
    pos_s = (np.asarray(inputs["clip_space_pos"], np.float32)[:, None, :]
             + np.asarray(inputs["clip_temporal_pos"], np.float32)[None, :, :]
             ).reshape(NT, D)
    pos_a = (np.asarray(inputs["audio_space_pos"], np.float32)[:, None, :]
             + np.asarray(inputs["audio_temporal_pos"], np.float32)[None, :, :]
             ).reshape(NT, D)
    qwT = np.ascontiguousarray(q_w.T).astype(bf)
    kvwT = np.ascontiguousarray(kv_w.T).astype(bf)
    projT = np.ascontiguousarray(proj_w.T).astype(bf)
    qb_nz = bool(np.any(q_b))
    kb_nz = bool(np.any(kv_b[:D]))
    vb_nz = bool(np.any(kv_b[D:]))
    pb_nz = bool(np.any(proj_b))

    key = (qb_nz, kb_nz, vb_nz, pb_nz)
    if key not in _CACHE:
        _CACHE[key] = _build_nc(*key)
    nc = _CACHE[key]

    shared = {"qwT": qwT, "kvwT": kvwT, "projT": projT}
    if qb_nz:
        shared["qb"] = np.ascontiguousarray(q_b.reshape(DC, P).T)
    if kb_nz:
        shared["kb"] = np.ascontiguousarray(kv_b[:D].reshape(DC, P).T)
    if vb_nz:
        shared["vb"] = np.ascontiguousarray(kv_b[D:].reshape(1, D)).astype(bf)
    if pb_nz:
        shared["pb"] = np.ascontiguousarray(proj_b.reshape(1, D)).astype(bf)

    in_maps = []
    for b in range(N_CORES):
        m = dict(shared)
        m["xsT"] = np.ascontiguousarray(
            (s_x[1:, b * T:(b + 1) * T, :].reshape(NT, D) + pos_s).T).astype(bf)
        m["xaT"] = np.ascontiguousarray(
            (audio[2:, b * T:(b + 1) * T, :].reshape(NT, D) + pos_a).T).astype(bf)
        in_maps.append(m)

    from concourse.bass_utils import run_bass_kernel_spmd
    res = run_bass_kernel_spmd(nc, in_maps, core_ids=list(range(N_CORES)))
    LAST["exec_time_ns"] = res.exec_time_ns
    LAST["trace"] = res.instructions_and_trace

    out_full = np.empty((2 + APATCH, B * T, D), np.float32)
    out_full[:2] = audio[:2]
    for b in range(N_CORES):
        out_full[2:, b * T:(b + 1) * T, :] = \
            res.results[b]["out"].reshape(APATCH, T, D)
    return out_full


# revision 39
# speedup vs baseline: 1.0596x; 1.0097x over previous
"""Cross-attention (S2Audio) Trainium2 Bass kernel.

Sharding: data-parallel over the clip batch B=8 -> one batch element per
NeuronCore.  Per core the kernel computes, for its batch element b:

  q = (audio_patch + pos_a) @ q_w.T + q_b          (1568, 768)
  k,v = (s_x_patch + pos_s) @ kv_w.T + kv_b        (1568, 768) each
  out = softmax(q k^T / sqrt(64)) v  per 12 heads  -> proj -> (1568, 768)

Host prep is layout/elementwise only: weight transposes, positional-embedding
combine + add (O(N*D)), bf16 casts, sharding slices.  All matmuls/softmax run
on device.

Performance-critical structure (v2):
  * The TRN2 PE clock-gates to 1.2 GHz (HAM K=4/8) whenever it idles; dense
    back-to-back matmul emission keeps it at 2.4 GHz.  All per-head serial
    work (softmax normalization) is OFF the PE queue: denominators come free
    from a ones-column in the PV matmul, reciprocals are batched per block on
    DVE ([12, nq] in one instruction), the partition-broadcast runs on the
    otherwise-idle GpSimd engine, and the final scale is an in-place DVE mul.
    The whole normalize + O-projection of block b-1 is software-pipelined
    into block b's head loop.
  * Scores matmuls have K=64 (head dim) -> 64x128 PE row tiling: the two
    heads of a pair live on SBUF partitions 0-63 / 64-127, their score
    matmuls are emitted interleaved (tile_position (0,0)/(64,0)) so they
    stream CONCURRENTLY through the two 64-row halves of the PE array.
  * Both heads' scores for a token chunk land in one 2-bank PSUM quad tile;
    a single ScalarE ACTIVATE [tw, 2*nq] applies exp to the pair (fused
    1/sqrt(64) scale, bf16 out) - ScalarE instruction count matters because
    exp is the attention-phase throughput floor.
  * PV of pair c-1 is emitted BEFORE scores of pair c so ready PE work never
    queues behind score matmuls that are gated on the exp pipeline.
  * Weight/activation DMAs are issued per-chunk, compute-first order, so the
    first K-proj matmul starts ~4us in and phase transitions have no PE gap.
"""

import numpy as np
from contextlib import ExitStack

B, T, NPATCH, APATCH, D, H = 8, 8, 196, 196, 768, 12
HD = D // H                      # 64
SCALE = float(HD) ** -0.5        # 0.125
NT = NPATCH * T                  # 1568 tokens (same count for q and kv side)
P = 128
DC = D // P                      # 6 feature chunks
N_CORES = 8

# token chunks (partition-dim tiling): 12 x 128 + 1 x 32
TOK_CHUNKS = [(i * P, min(P, NT - i * P)) for i in range((NT + P - 1) // P)]
NTC = len(TOK_CHUNKS)            # 13
# nq blocks for the attention/output stage.  The degenerate 32-query block
# goes first: its scores/exp are emitted inside phase 1 (slot-packed, one
# ACT per pair) and only its PV/normalize run in the main loop.
NQB = 512
NQ_BLOCKS = [(1536, 32), (0, 512), (512, 512), (1024, 512)]
NPAIR = H // 2                   # 6 head pairs

_CACHE: dict = {}
LAST: dict = {"exec_time_ns": None, "trace": None}


def _build_nc(qb_nz: bool, kb_nz: bool, vb_nz: bool, pb_nz: bool):
    import concourse.mybir as mybir
    from concourse import bacc
    from concourse.tile import TileContext

    f32 = mybir.dt.float32
    bf16 = mybir.dt.bfloat16
    AF = mybir.ActivationFunctionType

    nc = bacc.Bacc("TRN2", target_bir_lowering=False, debug=False,
                   num_devices=N_CORES)

    xsT = nc.dram_tensor("xsT", [D, NT], bf16, kind="ExternalInput")
    xaT = nc.dram_tensor("xaT", [D, NT], bf16, kind="ExternalInput")
    qwT = nc.dram_tensor("qwT", [D, D], bf16, kind="ExternalInput")
    kvwT = nc.dram_tensor("kvwT", [D, 2 * D], bf16, kind="ExternalInput")
    projT = nc.dram_tensor("projT", [D, D], bf16, kind="ExternalInput")
    qb = nc.dram_tensor("qb", [P, DC], f32, kind="ExternalInput") if qb_nz else None
    kb = nc.dram_tensor("kb", [P, DC], f32, kind="ExternalInput") if kb_nz else None
    vb = nc.dram_tensor("vb", [1, D], bf16, kind="ExternalInput") if vb_nz else None
    pb = nc.dram_tensor("pb", [1, D], bf16, kind="ExternalInput") if pb_nz else None
    out = nc.dram_tensor("out", [NT, D], f32, kind="ExternalOutput")

    with TileContext(nc) as tc, ExitStack() as ctx:
        consts = ctx.enter_context(tc.tile_pool(name="consts", bufs=1))
        persist = ctx.enter_context(tc.tile_pool(name="persist", bufs=1))

        ones_bf = consts.tile([1, P], bf16, tag="ones_bf")
        nc.gpsimd.memset(ones_bf[:], 1.0)
        qb_sb = kb_sb = vb_sb = pb_sb = None
        if qb_nz:
            qb_sb = consts.tile([P, DC], f32, tag="qb")
            nc.sync.dma_start(qb_sb[:], qb[:])
        if kb_nz:
            kb_sb = consts.tile([P, DC], f32, tag="kb")
            nc.sync.dma_start(kb_sb[:], kb[:])
        if vb_nz:
            vb_sb = consts.tile([1, D], bf16, tag="vb")
            nc.sync.dma_start(vb_sb[:], vb[:])
        if pb_nz:
            pb_sb = consts.tile([1, D], bf16, tag="pb")
            nc.sync.dma_start(pb_sb[:], pb[:])

        # persistent SBUF tensors: K (feature-major) and V (token-major)
        k_feat = [persist.tile([P, NT], bf16, tag=f"k_feat{c}", name=f"k_feat{c}")
                  for c in range(DC)]
        v_st = [persist.tile([P, H, HD + 1], bf16, tag=f"v{i}", name=f"v{i}")
                for i in range(NTC)]

        # phase-2 weights, prefetched during phase 1
        qw_sb = persist.tile([P, DC, D], bf16, tag="qw", name="qw")
        pw_sb = persist.tile([P, DC, D], bf16, tag="pw", name="pw")

        # phase-2 SBUF pools + the score-quad PSUM pool open before phase 1
        # so the degenerate 32-query block's Q-proj/scores/exp can be
        # emitted INSIDE phase 1 (ScalarE is otherwise idle there, and the
        # list scheduler will not hoist work across emission order when
        # phase 1 has no stalls).
        xfb = ctx.enter_context(tc.tile_pool(name="xfb", bufs=2))
        qfb = ctx.enter_context(tc.tile_pool(name="qfb", bufs=2))
        expp = ctx.enter_context(tc.tile_pool(name="expp", bufs=3))
        hexp = ctx.enter_context(tc.tile_pool(name="hexp", bufs=1))
        ofp = ctx.enter_context(tc.tile_pool(name="ofp", bufs=2))
        otp = ctx.enter_context(tc.tile_pool(name="otp", bufs=2))
        nrm = ctx.enter_context(tc.tile_pool(name="nrm", bufs=1))
        bcp = ctx.enter_context(tc.tile_pool(name="bcp", bufs=2))
        scq = ctx.enter_context(tc.tile_pool(name="scq", bufs=2, space="PSUM"))
        q_feat32 = [qfb.tile([P, 32], bf16, tag=f"qf32_{c}", name=f"qf32_{c}")
                    for c in range(DC)]
        hexp32 = [hexp.tile([P, 2, NTC * 32], bf16, tag=f"he{c}", name=f"he{c}")
                  for c in range(NPAIR)]

        # ---------------- phase 1: K and V projections ----------------
        with ExitStack() as ph:
            wtp = ph.enter_context(tc.tile_pool(name="wtp", bufs=1))
            xfp = ph.enter_context(tc.tile_pool(name="xfp", bufs=1))
            ps1 = ph.enter_context(tc.tile_pool(name="ps1", bufs=4, space="PSUM"))

            kvw_sb = wtp.tile([P, DC, 2 * D], bf16, tag="kvw", name="kvw")
            xs_feat = [xfp.tile([P, NT], bf16, tag=f"xsf{c}", name=f"xsf{c}")
                       for c in range(DC)]
            # compute-first DMA order: K-proj can start after the first
            # kvw/xs chunk pair lands; phase-2 weights stream in behind.
            # The first chunk's transfers are split so the very first
            # matmul group is ready sooner.
            # spread the startup loads over three hardware DMA queues
            # (sync / scalar / gpsimd triggers) so the first matmul and the
            # first exp are not gated on one serial queue.
            nc.sync.dma_start(kvw_sb[:, 0, :D], kvwT[0:P, :D])
            nc.sync.dma_start(xs_feat[0][:, :NQB], xsT[0:P, :NQB])
            nc.sync.dma_start(kvw_sb[:, 0, D:], kvwT[0:P, D:])
            nc.sync.dma_start(xs_feat[0][:, NQB:], xsT[0:P, NQB:])
            for c in range(1, DC):
                q = nc.sync if c % 2 else nc.gpsimd
                q.dma_start(kvw_sb[:, c, :], kvwT[c * P:(c + 1) * P, :])
                q.dma_start(xs_feat[c][:], xsT[c * P:(c + 1) * P, :])
            nc.scalar.dma_start(qw_sb[:],
                                qwT.rearrange("(c p) d -> p c d", p=P))
            nc.scalar.dma_start(pw_sb[:],
                                projT.rearrange("(c p) d -> p c d", p=P))

            # K projection (feature-major)
            for m in range(DC):
                for (n0, nw) in NQ_BLOCKS:
                    ps = ps1.tile([P, NQB], f32, tag="big", name="kproj")
                    for c in range(DC):
                        nc.tensor.matmul(ps[:, :nw],
                                         kvw_sb[:, c, m * P:(m + 1) * P],
                                         xs_feat[c][:, n0:n0 + nw],
                                         start=(c == 0), stop=(c == DC - 1))
                    if kb_nz:
                        nc.scalar.activation(k_feat[m][:, n0:n0 + nw],
                                             ps[:, :nw], AF.Identity,
                                             bias=kb_sb[:, m:m + 1])
                    else:
                        nc.vector.tensor_copy(k_feat[m][:, n0:n0 + nw],
                                              ps[:, :nw])

            # --- 32-query block prelude: Q-proj + slot-packed scores +
            # exp, emitted here so ScalarE has work during V-proj.  PV for
            # these runs after V-proj in the main block loop. ---
            xa32 = xfb.tile([P, DC, 32], bf16, tag="xa32", name="xa32")
            for c in range(DC):
                nc.sync.dma_start(xa32[:, c, :], xaT[c * P:(c + 1) * P, NT - 32:])
            for m in range(DC):
                ps = ps1.tile([P, NQB], f32, tag="big", name="q32proj")
                for c in range(DC):
                    nc.tensor.matmul(ps[:, :32],
                                     qw_sb[:, c, m * P:(m + 1) * P],
                                     xa32[:, c, :],
                                     start=(c == 0), stop=(c == DC - 1))
                if qb_nz:
                    nc.scalar.activation(q_feat32[m][:], ps[:, :32],
                                         AF.Identity, bias=qb_sb[:, m:m + 1])
                else:
                    nc.vector.tensor_copy(q_feat32[m][:], ps[:, :32])
            for c in range(NPAIR):
                qd = scq.tile([P, 2, NQB], f32, tag="quad", name="squad")
                for ti, (t0, tw) in enumerate(TOK_CHUNKS):
                    for par in range(2):
                        hp = par * HD
                        nc.tensor.matmul(
                            qd[:tw, par, ti * 32:(ti + 1) * 32],
                            k_feat[c][hp:hp + HD, t0:t0 + tw],
                            q_feat32[c][hp:hp + HD, :],
                            start=True, stop=True)
                nc.scalar.activation(hexp32[c][:, :, :NTC * 32],
                                     qd[:, :, :NTC * 32], AF.Exp, scale=SCALE)

            # V projection (token-major, interleaved with ones column)
            for ti, (t0, tw) in enumerate(TOK_CHUNKS):
                for half in range(2):
                    ps = ps1.tile([P, NQB], f32, tag="big", name="vproj")
                    for c in range(DC):
                        nc.tensor.matmul(
                            ps[:tw, :384],
                            xs_feat[c][:, t0:t0 + tw],
                            kvw_sb[:, c, D + half * 384:D + (half + 1) * 384],
                            start=(c == 0), stop=(c == DC - 1 and not vb_nz))
                    if vb_nz:
                        nc.tensor.matmul(
                            ps[:tw, :384], ones_bf[:, :tw],
                            vb_sb[:, half * 384:(half + 1) * 384],
                            start=False, stop=True)
                    nc.vector.tensor_copy(
                        v_st[ti][:tw, half * 6:(half + 1) * 6, :HD],
                        ps[:tw, :384].rearrange("p (h d) -> p h d", d=HD))
                nc.vector.memset(v_st[ti][:tw, :, HD:], 1.0)

        # -------- phase 2: per-block Q proj + attention + O-proj --------
        if True:
            pvps = ctx.enter_context(tc.tile_pool(name="pvps", bufs=2,
                                                  space="PSUM"))
            prj = ctx.enter_context(tc.tile_pool(name="prj", bufs=2,
                                                 space="PSUM"))

            def emit_attn_pair(c, blk, nw):
                """Attention for head pair (2c, 2c+1).  The two heads'
                score matmuls go to PE row tiles (0,0)/(64,0) back-to-back
                so they stream concurrently; one ScalarE ACT applies exp to
                the 2-bank quad; PV matmuls trail scores by one token chunk
                so PE and ScalarE both stay continuously fed.  Row 64 of
                each pv (from the V ones-column) is the softmax
                denominator."""
                q_feat = blk.get("q_feat")
                pvt = [pvps.tile([HD + 1, NQB], f32, tag="pv", name="pv")
                       for _ in range(2)]

                def pv_chunk(ti, et):
                    t0, tw = TOK_CHUNKS[ti]
                    for par in range(2):
                        nc.tensor.matmul(pvt[par][:, :nw],
                                         v_st[ti][:tw, 2 * c + par, :],
                                         et[:tw, par, :nw],
                                         start=(ti == 0),
                                         stop=(ti == NTC - 1))

                if "hexp" in blk:
                    # 32-query block: scores + exp were emitted inside
                    # phase 1 (slot-packed); only PV remains here.
                    et = blk["hexp"][c]
                    for ti, (t0, tw) in enumerate(TOK_CHUNKS):
                        for par in range(2):
                            nc.tensor.matmul(
                                pvt[par][:, :nw],
                                v_st[ti][:tw, 2 * c + par, :],
                                et[:tw, par, ti * nw:(ti + 1) * nw],
                                start=(ti == 0), stop=(ti == NTC - 1))
                else:
                    prev_et = None
                    for ti, (t0, tw) in enumerate(TOK_CHUNKS):
                        qd = scq.tile([P, 2, NQB], f32, tag="quad",
                                      name="squad")
                        for par in range(2):
                            hp = par * HD
                            nc.tensor.matmul(
                                qd[:tw, par, :nw],
                                k_feat[c][hp:hp + HD, t0:t0 + tw],
                                q_feat[c][hp:hp + HD, :nw],
                                start=True, stop=True)
                        et = expp.tile([P, 2, NQB], bf16, tag="exp",
                                       name="exp")
                        nc.scalar.activation(et[:tw, :, :nw],
                                             qd[:tw, :, :nw],
                                             AF.Exp, scale=SCALE)
                        if prev_et is not None:
                            pv_chunk(ti - 1, prev_et)
                        prev_et = et
                    pv_chunk(NTC - 1, prev_et)

                # drain: denominators free-major onto partition 0 (DVE APs
                # need 32-aligned partition bases, so a [12, nq] gather is
                # staged via DMA in norm_stage 0), numerators into out_feat
                # (pre-normalization; scaled in-place next block)
                for par in range(2):
                    h = 2 * c + par
                    nc.vector.tensor_copy(blk["den_st"][0:1, h, :nw],
                                          pvt[par][HD:HD + 1, :nw])
                    nc.vector.tensor_copy(
                        blk["out_feat"][c][par * HD:(par + 1) * HD, :nw],
                        pvt[par][:HD, :nw])

            def norm_stage(blk, stage):
                """Normalize + O-proj of a prior block, split into 6 stages
                interleaved into the successor block's head loop."""
                if blk is None:
                    return
                nw, n0 = blk["nw"], blk["n0"]
                if stage in (0, 1):
                    # partition_broadcast requires dst base partition 0 and
                    # tensor_tensor requires equal input bases: broadcast the
                    # odd head's reciprocal to all 128 partitions, overwrite
                    # partitions 0-63 with the even head's (gpsimd FIFO
                    # orders the writes), then one full-pair mul at base 0.
                    for c in range(3 * stage, 3 * (stage + 1)):
                        bc = bcp.tile([P, NQB], bf16, tag="bc", name="bc")
                        nc.gpsimd.partition_broadcast(
                            bc[:, :nw], blk["rec_st"][0:1, 2 * c + 1, :nw])
                        nc.gpsimd.partition_broadcast(
                            bc[:HD, :nw], blk["rec_st"][0:1, 2 * c, :nw])
                        nc.vector.tensor_mul(blk["out_feat"][c][:, :nw],
                                             blk["out_feat"][c][:, :nw],
                                             bc[:, :nw])
                elif stage in (2, 3):
                    # O-projection chunk groups + output DMA
                    chunks = [(cc, min(P, nw - cc)) for cc in range(0, nw, P)]
                    lo = (stage - 2) * 2
                    for (c0, cw) in chunks[lo:lo + 2]:
                        ot = otp.tile([P, D], f32, tag="ot", name="ot")
                        for half in range(2):
                            ps = prj.tile([P, NQB], f32, tag="prj", name="oproj")
                            for c in range(DC):
                                nc.tensor.matmul(
                                    ps[:cw, :384],
                                    blk["out_feat"][c][:, c0:c0 + cw],
                                    pw_sb[:, c, half * 384:(half + 1) * 384],
                                    start=(c == 0),
                                    stop=(c == DC - 1 and not pb_nz))
                            if pb_nz:
                                nc.tensor.matmul(
                                    ps[:cw, :384], ones_bf[:, :cw],
                                    pb_sb[:, half * 384:(half + 1) * 384],
                                    start=False, stop=True)
                            nc.vector.tensor_copy(
                                ot[:cw, half * 384:(half + 1) * 384],
                                ps[:cw, :384])
                        nc.sync.dma_start(out[n0 + c0:n0 + c0 + cw, :],
                                          ot[:cw, :])

            prev = None
            for bi, (n0, nw) in enumerate(NQ_BLOCKS):
                blk = {"n0": n0, "nw": nw}
                is_last = bi == len(NQ_BLOCKS) - 1
                if bi == 0:
                    blk["hexp"] = hexp32
                else:
                    # load + project Q for this block (feature-major)
                    xa_feat = [xfb.tile([P, NQB], bf16, tag=f"xaf{c}",
                                        name=f"xaf{c}") for c in range(DC)]
                    for c in range(DC):
                        nc.sync.dma_start(xa_feat[c][:, :nw],
                                          xaT[c * P:(c + 1) * P, n0:n0 + nw])
                    q_feat = [qfb.tile([P, NQB], bf16, tag=f"qf{c}",
                                       name=f"qf{c}") for c in range(DC)]
                    for m in range(DC):
                        ps = prj.tile([P, NQB], f32, tag="prj", name="qproj")
                        for c in range(DC):
                            nc.tensor.matmul(ps[:, :nw],
                                             qw_sb[:, c, m * P:(m + 1) * P],
                                             xa_feat[c][:, :nw],
                                             start=(c == 0),
                                             stop=(c == DC - 1))
                        if qb_nz:
                            nc.scalar.activation(q_feat[m][:, :nw],
                                                 ps[:, :nw], AF.Identity,
                                                 bias=qb_sb[:, m:m + 1])
                        else:
                            nc.vector.tensor_copy(q_feat[m][:, :nw],
                                                  ps[:, :nw])
                    blk["q_feat"] = q_feat
                blk["out_feat"] = [ofp.tile([P, NQB], bf16, tag=f"of{c}",
                                            name=f"of{c}") for c in range(DC)]
                blk["den_st"] = nrm.tile([1, H, NQB], bf16, tag="denst",
                                         name="den_st")
                blk["den12"] = nrm.tile([H, NQB], bf16, tag="den",
                                        name="den12")
                blk["rec12"] = nrm.tile([H, NQB], f32, tag="rec", name="rec12")
                blk["rec_st"] = nrm.tile([1, H, NQB], bf16, tag="recst",
                                         name="rec_st", bufs=2)
                blk["rec12b"] = nrm.tile([H, NQB], bf16, tag="recb",
                                         name="rec12b")

                for c in range(NPAIR):
                    emit_attn_pair(c, blk, nw)
                    norm_stage(prev, c)
                    if is_last:
                        # final block: normalize per pair inline so the
                        # end-of-kernel serial chain is one pair deep, not
                        # a whole block.  Same DMA round-trip as below but
                        # on a [2, nq] slice (32-aligned bases via pair
                        # staging tiles at partition 0).
                        dp = nrm.tile([2, NQB], bf16, tag="dpair",
                                      name="dpair", bufs=2)
                        rp = nrm.tile([2, NQB], f32, tag="rpair",
                                      name="rpair", bufs=2)
                        nc.sync.dma_start(
                            dp[:, :nw],
                            blk["den_st"][0:1, 2 * c:2 * c + 2, :nw])
                        nc.vector.reciprocal(rp[:, :nw], dp[:, :nw])
                        rpb = nrm.tile([2, NQB], bf16, tag="rpairb",
                                       name="rpairb", bufs=2)
                        nc.vector.tensor_copy(rpb[:, :nw], rp[:, :nw])
                        nc.sync.dma_start(
                            blk["rec_st"][0:1, 2 * c:2 * c + 2, :nw],
                            rpb[:, :nw])
                        bc = bcp.tile([P, NQB], bf16, tag="bc", name="bc")
                        nc.gpsimd.partition_broadcast(
                            bc[:, :nw], blk["rec_st"][0:1, 2 * c + 1, :nw])
                        nc.gpsimd.partition_broadcast(
                            bc[:HD, :nw], blk["rec_st"][0:1, 2 * c, :nw])
                        nc.vector.tensor_mul(blk["out_feat"][c][:, :nw],
                                             blk["out_feat"][c][:, :nw],
                                             bc[:, :nw])
                if is_last:
                    norm_stage(blk, 2)
                    norm_stage(blk, 3)
                else:
                    # gather the 12 denominators to [12, nq] partitions via
                    # DMA (engine APs need 32-aligned partition bases; DMA
                    # does not), one batched DVE reciprocal, then scatter
                    # back free-major for the gpsimd partition_broadcast
                    # reads next block.
                    nc.sync.dma_start(blk["den12"][:, :nw],
                                      blk["den_st"][0:1, :, :nw])
                    nc.vector.reciprocal(blk["rec12"][:, :nw],
                                         blk["den12"][:, :nw])
                    nc.vector.tensor_copy(blk["rec12b"][:, :nw],
                                          blk["rec12"][:, :nw])
                    nc.sync.dma_start(blk["rec_st"][0:1, :, :nw],
                                      blk["rec12b"][:, :nw])
                prev = blk

    nc.finalize()
    return nc


def kernel(**inputs) -> np.ndarray:
    import ml_dtypes
    bf = ml_dtypes.bfloat16

    s_x = np.asarray(inputs["s_x"], np.float32)
    audio = np.asarray(inputs["audio"], np.float32)
    q_w = np.asarray(inputs["q_w"], np.float32)
    q_b = np.asarray(inputs["q_b"], np.float32)
    kv_w = np.asarray(inputs["kv_w"], np.float32)
    kv_b = np.asarray(inputs["kv_b"], np.float32)
    proj_w = np.asarray(inputs["proj_w"], np.float32)
    proj_b = np.asarray(inputs["proj_b"], np.float32)

    # host prep: layout + O(N*D) positional add + bf16 casts only
    pos_s = (np.asarray(inputs["clip_space_pos"], np.float32)[:, None, :]
             + np.asarray(inputs["clip_temporal_pos"], np.float32)[None, :, :]
             ).reshape(NT, D)
    pos_a = (np.asarray(inputs["audio_space_pos"], np.float32)[:, None, :]
             + np.asarray(inputs["audio_temporal_pos"], np.float32)[None, :, :]
             ).reshape(NT, D)
    qwT = np.ascontiguousarray(q_w.T).astype(bf)
    kvwT = np.ascontiguousarray(kv_w.T).astype(bf)
    projT = np.ascontiguousarray(proj_w.T).astype(bf)
    qb_nz = bool(np.any(q_b))
    kb_nz = bool(np.any(kv_b[:D]))
    vb_nz = bool(np.any(kv_b[D:]))
    pb_nz = bool(np.any(proj_b))

    key = (qb_nz, kb_nz, vb_nz, pb_nz)
    if key not in _CACHE:
        _CACHE[key] = _build_nc(*key)
    nc = _CACHE[key]

    shared = {"qwT": qwT, "kvwT": kvwT, "projT": projT}
    if qb_nz:
        shared["qb"] = np.ascontiguousarray(q_b.reshape(DC, P).T)
    if kb_nz:
        shared["kb"] = np.ascontiguousarray(kv_b[:D].reshape(DC, P).T)
    if vb_nz:
        shared["vb"] = np.ascontiguousarray(kv_b[D:].reshape(1, D)).astype(bf)
    if pb_nz:
        shared["pb"] = np.ascontiguousarray(proj_b.reshape(1, D)).astype(bf)

    in_maps = []
    for b in range(N_CORES):
        m = dict(shared)
        m["xsT"] = np.ascontiguousarray(
            (s_x[1:, b * T:(b + 1) * T, :].reshape(NT, D) + pos_s).T).astype(bf)
        m["xaT"] = np.ascontiguousarray(
            (audio[2:, b * T:(b + 1) * T, :].reshape(NT, D) + pos_a).T).astype(bf)
        in_maps.append(m)

    from concourse.bass_utils import run_bass_kernel_spmd
    res = run_bass_kernel_spmd(nc, in_maps, core_ids=list(range(N_CORES)))
    LAST["exec_time_ns"] = res.exec_time_ns
    LAST["trace"] = res.instructions_and_trace

    out_full = np.empty((2 + APATCH, B * T, D), np.float32)
    out_full[:2] = audio[:2]
    for b in range(N_CORES):
        out_full[2:, b * T:(b + 1) * T, :] = \
            res.results[b]["out"].reshape(APATCH, T, D)
    return out_full
